# revision 16
# baseline (speedup 1.0000x reference)
"""Bilinear interpolation sampling kernel for Trainium2 (8 NeuronCores).

Strategy (see spec sharding_hint): shard the feature_map over episodes B
(8 episodes per core) and route each agent's points to the core owning its
episode. Per core, fully on-device:
  Phase A: pad (+oom border) and transpose the 8 owned episode feature maps
           from [CE, 100, 100] to channels-last [102*102, CE] in DRAM
           (PE transpose through PSUM), stored as 4 episode-pair tables.
  Phase B: per episode-pair segment, compute mapped coords / floor / ceil /
           clip / bilinear weights / int16 row indices on DVE in one pass,
           then per 2048-point chunk gather row-pairs (2 taps x 128ch = 1KB
           contiguous) with dma_gather, apply the 4 bilinear weights
           (broadcast APs, in place) and write output rows.
Host side only shards/permutes inputs, pads to a shared SPMD schedule, and
scatters result rows back.
"""
import functools
import numpy as np

A, TD, B, CE = 4096, 60, 64, 128
NCORES = 8
EPC = B // NCORES          # episodes per core (8)
NSEG = EPC // 2            # one segment per episode pair (4)
HP = 102                   # padded grid
ROWS_EP = HP * HP          # 10404
ROWS_PAIR = 2 * ROWS_EP    # 20808
CHUNK = 2048               # points per gather chunk (16 slots of 128)
CSL = CHUNK // 128         # slots per chunk (16)
R112 = float(np.float32(1.0) / np.float32(112.0))


def _install_axon_profile_hook():
    """Register antenv.axon_hooks if the image lacks it, so trace=True (or a
    harness-set BASS_TRACE) can capture NTFF profiles instead of degrading."""
    import sys, types
    if 'antenv.axon_hooks' in sys.modules:
        return
    try:
        from antenv.axon_hooks import get_axon_ntff_profile_hook  # noqa: F401
        return
    except ImportError:
        pass
    try:
        from trn_agent_boot.trn_boot import _ntff_profile_via_ctypes
        hook = _ntff_profile_via_ctypes('/opt/axon/libaxon_pjrt.so')
    except Exception:
        hook = None
    mod = types.ModuleType('antenv.axon_hooks')
    mod.get_axon_ntff_profile_hook = lambda: hook
    mod.set_axon_ntff_profile_hook = lambda h: None
    sys.modules['antenv.axon_hooks'] = mod


def _chunks_of(seg_pts):
    out = []
    r = seg_pts
    while r > 0:
        c = min(r, CHUNK)
        out.append(c)
        r -= c
    return out


def _build_program(seg_pts, oomv):
    import concourse.bass as bass
    import concourse.mybir as mybir
    import concourse.tile as tile
    from concourse import bacc
    from concourse.masks import make_identity

    f32 = mybir.dt.float32
    i32 = mybir.dt.int32
    i16 = mybir.dt.int16
    OP = mybir.AluOpType

    npad = int(sum(seg_pts))
    nc = bacc.Bacc("TRN2", target_bir_lowering=False, debug=False,
                   num_devices=NCORES)

    fm_in = nc.dram_tensor("fm", [EPC, CE, 10000], f32, kind="ExternalInput").ap()
    seq_in = nc.dram_tensor("seqw", [npad, 2], f32, kind="ExternalInput").ap()
    c16_in = nc.dram_tensor("c16", [8 * npad, 3], f32, kind="ExternalInput").ap()
    out_feat = nc.dram_tensor("out_feat", [npad, CE], f32, kind="ExternalOutput").ap()
    out_mc = nc.dram_tensor("out_mc", [npad, 2], f32, kind="ExternalOutput").ap()

    with tile.TileContext(nc) as tc:
        with tc.tile_pool(name="dram", bufs=1, space="DRAM") as dpool, \
             tc.tile_pool(name="const", bufs=1) as cpool, \
             tc.tile_pool(name="tpose", bufs=2) as apool, \
             tc.tile_pool(name="psum", bufs=4, space="PSUM") as ppool, \
             tc.tile_pool(name="pts", bufs=2) as bpool, \
             tc.tile_pool(name="taps", bufs=2) as tpool, \
             tc.tile_pool(name="prod", bufs=2) as prpool:

            ident = cpool.tile([128, 128], f32)
            make_identity(nc, ident[:])
            ztile = cpool.tile([128, 128], f32)
            nc.vector.memset(ztile[:], oomv)

            fmp = [dpool.tile([ROWS_PAIR, CE], f32, tag=f"fmp{s}",
                              name=f"fmp{s}")
                   for s in range(NSEG)]

            def phase_a(s):
                # ---- Phase A: borders + transpose the segment's 2 episodes
                fs = fmp[s][:]
                for e01 in range(2):
                    eoff = e01 * ROWS_EP
                    nc.scalar.dma_start(out=fs[eoff: eoff + HP, :],
                                        in_=ztile[:HP, :])
                    nc.scalar.dma_start(
                        out=fs[eoff + 101 * HP + 1: eoff + 101 * HP + 102, :],
                        in_=ztile[:101, :])
                    col0 = bass.AP(fs.tensor, fs.offset + (eoff + HP) * CE,
                                   [[HP * CE, 101], [1, CE]])
                    nc.scalar.dma_start(out=col0, in_=ztile[:101, :])
                    col = bass.AP(fs.tensor, fs.offset + (eoff + HP + 101) * CE,
                                  [[HP * CE, 100], [1, CE]])
                    nc.scalar.dma_start(out=col, in_=ztile[:100, :])
                for e01 in range(2):
                    ep = 2 * s + e01
                    eoff = e01 * ROWS_EP
                    for blk in range(10):
                        it = apool.tile([128, 1000], f32, tag="tin")
                        nc.scalar.dma_start(
                            out=it[:], in_=fm_in[ep, :, blk * 1000:(blk + 1) * 1000])
                        ot = apool.tile([100, 10 * CE], f32, tag="tout")
                        for g, nb in ((0, 4), (4, 4), (8, 2)):
                            ps = ppool.tile([100, 512], f32, tag="ps")
                            for j in range(nb):
                                nc.tensor.transpose(
                                    out=ps[:, j * CE:(j + 1) * CE],
                                    in_=it[:, (g + j) * 100:(g + j + 1) * 100],
                                    identity=ident[:])
                            nc.scalar.copy(out=ot[:, g * CE:(g + nb) * CE],
                                           in_=ps[:, :nb * CE])
                        dst = bass.AP(
                            fs.tensor,
                            fs.offset + (eoff + (1 + blk * 10) * HP + 1) * CE,
                            [[CE, 100], [HP * CE, 10], [1, CE]])
                        nc.scalar.dma_start(out=dst, in_=ot[:].rearrange(
                            "p (r c) -> p r c", c=CE))

            seg_base = [0]
            for s in range(NSEG):
                seg_base.append(seg_base[-1] + seg_pts[s])

            seg_state = {}

            def phase_p(s):
                base = seg_base[s]
                # ---- Phase B1: coordinate/weight/index pass for the segment
                nt = seg_pts[s] // 128          # total slots in segment
                nt8 = 8 * nt
                sq = bpool.tile([128, nt, 2], f32, tag="sq")
                nc.sync.dma_start(
                    out=sq[:],
                    in_=seq_in[base:base + seg_pts[s], :].rearrange(
                        "(p k) c -> p k c", p=128))
                ct = bpool.tile([128, nt8, 3], f32, tag="ct")
                nc.sync.dma_start(
                    out=ct[:],
                    in_=c16_in[8 * base:8 * (base + seg_pts[s]), :].rearrange(
                        "(p t) c -> p t c", p=128))

                def mapped(src, tag, shape):
                    m = bpool.tile(shape, f32, tag=tag + "m")
                    nc.vector.tensor_scalar(out=m[:], in0=src, scalar1=56.0,
                                            scalar2=None, op0=OP.add)
                    nc.vector.tensor_scalar(out=m[:], in0=m[:], scalar1=R112,
                                            scalar2=100.0, op0=OP.mult,
                                            op1=OP.mult)
                    nc.vector.tensor_scalar(out=m[:], in0=m[:], scalar1=1.0,
                                            scalar2=None, op0=OP.add)
                    return m

                def floor_of(m, tag, shape):
                    ii = bpool.tile(shape, i32, tag=tag + "i")
                    nc.vector.tensor_copy(out=ii[:], in_=m[:])
                    ff = bpool.tile(shape, f32, tag=tag + "f")
                    nc.vector.tensor_copy(out=ff[:], in_=ii[:])
                    gt = bpool.tile(shape, f32, tag=tag + "g")
                    nc.vector.tensor_tensor(out=gt[:], in0=ff[:], in1=m[:],
                                            op=OP.is_gt)
                    nc.vector.tensor_tensor(out=ff[:], in0=ff[:], in1=gt[:],
                                            op=OP.subtract)
                    return ff

                def clip_to(v, hi, tag, shape):
                    nc.vector.tensor_scalar(out=v[:], in0=v[:], scalar1=0.0,
                                            scalar2=hi, op0=OP.max, op1=OP.min)
                    return v

                # ---- index pipeline on replicated [128, 8*nt] tiles
                sh8 = [128, nt8]
                xm16 = mapped(ct[:, :, 0], "x16", sh8)
                ym16 = mapped(ct[:, :, 1], "y16", sh8)
                # idx clip to [0,100] differs from weight clip [0,101] only
                # for exact-integer coords 101 where all 4 weights are 0.
                x1f16 = clip_to(floor_of(xm16, "fx16", sh8), 100.0, "cx16", sh8)
                y1f16 = clip_to(floor_of(ym16, "fy16", sh8), 100.0, "cy16", sh8)
                nc.vector.scalar_tensor_tensor(
                    out=y1f16[:], in0=y1f16[:], scalar=float(HP),
                    in1=x1f16[:], op0=OP.mult, op1=OP.add)
                nc.vector.scalar_tensor_tensor(
                    out=y1f16[:], in0=ct[:, :, 2], scalar=float(ROWS_EP),
                    in1=y1f16[:], op0=OP.mult, op1=OP.add)
                idx1 = bpool.tile(sh8, i16, tag="idx1", bufs=4)
                nc.vector.tensor_copy(out=idx1[:], in_=y1f16[:])

                # ---- weight pipeline on [128, nt] tiles
                shw = [128, nt]
                xm = mapped(sq[:, :, 0], "xw", shw)
                ym = mapped(sq[:, :, 1], "yw", shw)
                flx = floor_of(xm, "fxw", shw)
                fly = floor_of(ym, "fyw", shw)

                def ceil_of(fl, m, tag):
                    lt = bpool.tile(shw, f32, tag=tag + "t")
                    nc.vector.tensor_tensor(out=lt[:], in0=fl[:], in1=m[:],
                                            op=OP.is_lt)
                    nc.vector.tensor_tensor(out=lt[:], in0=fl[:], in1=lt[:],
                                            op=OP.add)
                    return lt

                cex = ceil_of(flx, xm, "cex")
                cey = ceil_of(fly, ym, "cey")
                x1c = clip_to(flx, 101.0, "x1w", shw)
                x2c = clip_to(cex, 101.0, "x2w", shw)
                y1c = clip_to(fly, 101.0, "y1w", shw)
                y2c = clip_to(cey, 101.0, "y2w", shw)

                def sub(p, q, tag):
                    o = bpool.tile(shw, f32, tag=tag)
                    nc.vector.tensor_tensor(out=o[:], in0=p[:], in1=q[:],
                                            op=OP.subtract)
                    return o

                wx1 = sub(x2c, xm, "wx1")   # (x2 - x)
                wx2 = sub(xm, x1c, "wx2")   # (x - x1)
                wy1 = sub(y2c, ym, "wy1")   # (y2 - y)
                wy2 = sub(ym, y1c, "wy2")   # (y - y1)

                # reference pairing: tap(y1,x1)*w11, tap(y1,x2)*w12,
                #                    tap(y2,x1)*w21, tap(y2,x2)*w22
                W1 = bpool.tile([128, nt, 2], f32, tag="W1", bufs=4)
                W2 = bpool.tile([128, nt, 2], f32, tag="W2", bufs=4)
                nc.vector.tensor_tensor(out=W1[:, :, 0], in0=wx1[:],
                                        in1=wy1[:], op=OP.mult)   # w11
                nc.vector.tensor_tensor(out=W1[:, :, 1], in0=wx1[:],
                                        in1=wy2[:], op=OP.mult)   # w12
                nc.vector.tensor_tensor(out=W2[:, :, 0], in0=wx2[:],
                                        in1=wy1[:], op=OP.mult)   # w21
                nc.vector.tensor_tensor(out=W2[:, :, 1], in0=wx2[:],
                                        in1=wy2[:], op=OP.mult)   # w22

                mc = bpool.tile([128, nt, 2], f32, tag="mc")
                nc.vector.tensor_copy(out=mc[:, :, 0], in_=xm[:])
                nc.vector.tensor_copy(out=mc[:, :, 1], in_=ym[:])
                nc.sync.dma_start(
                    out=out_mc[base:base + seg_pts[s], :].rearrange(
                        "(p k) c -> p k c", p=128),
                    in_=mc[:])

                seg_state[s] = (idx1, W1, W2, nt)

            def phase_g(s):
                base = seg_base[s]
                fs = fmp[s][:]
                idx1, W1, W2, nt = seg_state.pop(s)
                # ---- Phase B2: chunked gathers + weighted sum
                in_ap1 = bass.AP(fs.tensor, fs.offset,
                                 [[CE, ROWS_PAIR - 1], [1, 2 * CE]])
                in_ap2 = bass.AP(fs.tensor, fs.offset + HP * CE,
                                 [[CE, ROWS_PAIR - HP - 1], [1, 2 * CE]])
                for ci, csz in enumerate(_chunks_of(seg_pts[s])):
                    n = csz // 128
                    k0 = ci * CSL
                    T1 = tpool.tile([128, n, 2 * CE], f32, tag="T1")
                    nc.gpsimd.dma_gather(
                        out_ap=T1[:], in_ap=in_ap1,
                        idxs_ap=idx1[:, ci * 128: ci * 128 + 8 * n],
                        num_idxs=csz, num_idxs_reg=csz,
                        elem_size=2 * CE, elem_step=CE, single_packet=False)
                    T2 = tpool.tile([128, n, 2 * CE], f32, tag="T2")
                    nc.gpsimd.dma_gather(
                        out_ap=T2[:], in_ap=in_ap2,
                        idxs_ap=idx1[:, ci * 128: ci * 128 + 8 * n],
                        num_idxs=csz, num_idxs_reg=csz,
                        elem_size=2 * CE, elem_step=CE, single_packet=False)

                    def wbroad(W):
                        w = W[:, k0:k0 + n, :]
                        return bass.AP(w.tensor, w.offset,
                                       [list(w.ap[0]), [2, n], [1, 2], [0, CE]])

                    T1v = T1[:].rearrange("p k (h c) -> p k h c", h=2)
                    T2v = T2[:].rearrange("p k (h c) -> p k h c", h=2)
                    nc.vector.tensor_tensor(out=T1v, in0=T1v, in1=wbroad(W1),
                                            op=OP.mult)
                    nc.vector.tensor_tensor(out=T2v, in0=T2v, in1=wbroad(W2),
                                            op=OP.mult)
                    nc.vector.tensor_tensor(out=T1v, in0=T1v, in1=T2v,
                                            op=OP.add)
                    of = prpool.tile([128, n, CE], f32, tag="of")
                    nc.vector.tensor_tensor(out=of[:], in0=T1v[:, :, 0, :],
                                            in1=T1v[:, :, 1, :], op=OP.add)
                    # row(p, k) = base + p*nt + k0 + k
                    dst = bass.AP(out_feat.tensor,
                                  out_feat.offset + (base + k0) * CE,
                                  [[nt * CE, 128], [CE, n], [1, CE]])
                    nc.sync.dma_start(out=dst, in_=of[:])

            # schedule: transposes and coordinate pipelines run ahead of
            # the serial gather stream so the Pool engine never starves
            phase_a(0)
            phase_p(0)
            phase_a(1)
            phase_p(1)
            phase_g(0)
            phase_a(2)
            phase_p(2)
            phase_g(1)
            phase_a(3)
            phase_p(3)
            phase_g(2)
            phase_g(3)

    nc.compile()
    return nc


@functools.lru_cache(maxsize=4)
def _compiled(seg_pts, oomv):
    return _build_program(seg_pts, oomv)


def kernel(episode_idx, sequence, feature_map, oom_val):
    _install_axon_profile_hook()
    from concourse.bass_utils import run_bass_kernel_spmd

    ep = np.asarray(episode_idx).astype(np.int64)           # [A]
    seq = np.ascontiguousarray(np.asarray(sequence, dtype=np.float32))
    fm = np.ascontiguousarray(np.asarray(feature_map, dtype=np.float32))
    oomv = float(np.float32(oom_val))
    a_total, td = seq.shape[0], seq.shape[1]

    core_of = ep // EPC
    seg_of = (ep % EPC) // 2
    lep_of = (ep % 2).astype(np.float32)

    agents = [[[] for _ in range(NSEG)] for _ in range(NCORES)]
    for a in range(a_total):
        agents[int(core_of[a])][int(seg_of[a])].append(a)

    seg_pts = []
    for s in range(NSEG):
        mx = max(len(agents[c][s]) * td for c in range(NCORES))
        mx = max(mx, 1)
        seg_pts.append(int(-(-mx // 128) * 128))
    seg_pts = tuple(seg_pts)
    npad = sum(seg_pts)

    nc = _compiled(seg_pts, oomv)

    tdr = np.arange(td, dtype=np.int64)
    in_maps = []
    srcs = []
    for c in range(NCORES):
        rx = np.zeros(npad, np.float32)
        ry = np.zeros(npad, np.float32)
        rl = np.zeros(npad, np.float32)
        rsrc = np.full(npad, -1, np.int64)
        base = 0
        for s in range(NSEG):
            ags = np.asarray(agents[c][s], dtype=np.int64)
            npts = len(ags) * td
            if npts:
                rx[base:base + npts] = seq[ags, :, 0].ravel()
                ry[base:base + npts] = seq[ags, :, 1].ravel()
                rl[base:base + npts] = np.repeat(lep_of[ags], td)
                rsrc[base:base + npts] = (ags[:, None] * td + tdr[None, :]).ravel()
            base += seg_pts[s]
        # gather-order (16-wrap, replicated) blocks per segment
        c16_parts = []
        base = 0
        for s in range(NSEG):
            spts = seg_pts[s]
            nt = spts // 128
            i = np.arange(spts, dtype=np.int64)      # segment-linear gather order
            cidx = i // CHUNK
            j = i % CHUNK
            srow = base + (j % 128) * nt + cidx * CSL + (j // 128)
            blk = np.stack([rx[srow], ry[srow], rl[srow]], axis=1)   # [spts,3]
            t16 = blk.reshape(spts // 16, 16, 3).transpose(1, 0, 2)  # [16,8nt,3]
            c16_parts.append(np.tile(t16, (8, 1, 1)).reshape(-1, 3))
            base += spts
        in_maps.append({
            "fm": fm[c * EPC:(c + 1) * EPC].reshape(EPC, CE, 10000),
            "seqw": np.stack([rx, ry], axis=1),
            "c16": np.ascontiguousarray(np.concatenate(c16_parts, axis=0)),
        })
        srcs.append(rsrc)

    res = run_bass_kernel_spmd(nc, in_maps, core_ids=list(range(NCORES)))

    lf = np.zeros((a_total * td, CE), np.float32)
    mc = np.zeros((a_total * td, 2), np.float32)
    for c in range(NCORES):
        rsrc = srcs[c]
        v = rsrc >= 0
        lf[rsrc[v]] = res.results[c]["out_feat"][v]
        mc[rsrc[v]] = res.results[c]["out_mc"][v]
    return lf.reshape(a_total, td, CE), mc.reshape(a_total, td, 2)


# revision 17
# speedup vs baseline: 1.1347x; 1.1347x over previous
"""Bilinear interpolation sampling kernel for Trainium2 (8 NeuronCores).

Strategy (see spec sharding_hint): shard the feature_map over episodes B
(8 episodes per core) and route each agent's points to the core owning its
episode. Per core, fully on-device:
  Phase A: pad (+oom border) and transpose the 8 owned episode feature maps
           from [CE, 100, 100] to channels-last [102*102, CE] in DRAM
           (PE transpose through PSUM), stored as 4 episode-pair tables.
  Phase B: per episode-pair segment, compute mapped coords / floor / ceil /
           clip / bilinear weights / int16 row indices on DVE in one pass,
           then per 2048-point chunk gather row-pairs (2 taps x 128ch = 1KB
           contiguous) with dma_gather, apply the 4 bilinear weights
           (broadcast APs, in place) and write output rows.
Host side only shards/permutes inputs, pads to a shared SPMD schedule, and
scatters result rows back.
"""
import functools
import numpy as np

A, TD, B, CE = 4096, 60, 64, 128
NCORES = 8
EPC = B // NCORES          # episodes per core (8)
NSEG = EPC // 2            # one segment per episode pair (4)
HP = 102                   # padded grid
ROWS_EP = HP * HP          # 10404
ROWS_PAIR = 2 * ROWS_EP    # 20808
CHUNK = 2048               # points per gather chunk (16 slots of 128)
CSL = CHUNK // 128         # slots per chunk (16)
R112 = float(np.float32(1.0) / np.float32(112.0))


def _install_axon_profile_hook():
    """Register antenv.axon_hooks if the image lacks it, so trace=True (or a
    harness-set BASS_TRACE) can capture NTFF profiles instead of degrading."""
    import sys, types
    if 'antenv.axon_hooks' in sys.modules:
        return
    try:
        from antenv.axon_hooks import get_axon_ntff_profile_hook  # noqa: F401
        return
    except ImportError:
        pass
    try:
        from trn_agent_boot.trn_boot import _ntff_profile_via_ctypes
        hook = _ntff_profile_via_ctypes('/opt/axon/libaxon_pjrt.so')
    except Exception:
        hook = None
    mod = types.ModuleType('antenv.axon_hooks')
    mod.get_axon_ntff_profile_hook = lambda: hook
    mod.set_axon_ntff_profile_hook = lambda h: None
    sys.modules['antenv.axon_hooks'] = mod


def _chunks_of(seg_pts):
    out = []
    r = seg_pts
    while r > 0:
        c = min(r, CHUNK)
        out.append(c)
        r -= c
    return out


def _build_program(seg_pts, oomv):
    import concourse.bass as bass
    import concourse.mybir as mybir
    import concourse.tile as tile
    from concourse import bacc
    from concourse.masks import make_identity

    f32 = mybir.dt.float32
    i32 = mybir.dt.int32
    i16 = mybir.dt.int16
    OP = mybir.AluOpType

    npad = int(sum(seg_pts))
    nc = bacc.Bacc("TRN2", target_bir_lowering=False, debug=False,
                   num_devices=NCORES)

    fm_in = nc.dram_tensor("fm", [EPC, CE, 10000], f32, kind="ExternalInput").ap()
    seq_in = nc.dram_tensor("seqw", [npad, 2], f32, kind="ExternalInput").ap()
    c16_in = nc.dram_tensor("c16", [8 * npad, 3], f32, kind="ExternalInput").ap()
    out_feat = nc.dram_tensor("out_feat", [npad, CE], f32, kind="ExternalOutput").ap()
    out_mc = nc.dram_tensor("out_mc", [npad, 2], f32, kind="ExternalOutput").ap()

    with tile.TileContext(nc) as tc:
        with tc.tile_pool(name="dram", bufs=1, space="DRAM") as dpool, \
             tc.tile_pool(name="const", bufs=1) as cpool, \
             tc.tile_pool(name="tpose", bufs=2) as apool, \
             tc.tile_pool(name="psum", bufs=4, space="PSUM") as ppool, \
             tc.tile_pool(name="pts", bufs=2) as bpool, \
             tc.tile_pool(name="taps", bufs=3) as tpool, \
             tc.tile_pool(name="prod", bufs=2) as prpool:

            ident = cpool.tile([128, 128], f32)
            make_identity(nc, ident[:])
            ztile = cpool.tile([128, 128], f32)
            nc.vector.memset(ztile[:], oomv)

            fmp = [dpool.tile([ROWS_PAIR, CE], f32, tag=f"fmp{s}",
                              name=f"fmp{s}")
                   for s in range(NSEG)]

            def phase_a(s):
                # ---- Phase A: borders + transpose the segment's 2 episodes
                fs = fmp[s][:]
                for e01 in range(2):
                    eoff = e01 * ROWS_EP
                    nc.scalar.dma_start(out=fs[eoff: eoff + HP, :],
                                        in_=ztile[:HP, :])
                    nc.scalar.dma_start(
                        out=fs[eoff + 101 * HP + 1: eoff + 101 * HP + 102, :],
                        in_=ztile[:101, :])
                    col0 = bass.AP(fs.tensor, fs.offset + (eoff + HP) * CE,
                                   [[HP * CE, 101], [1, CE]])
                    nc.scalar.dma_start(out=col0, in_=ztile[:101, :])
                    col = bass.AP(fs.tensor, fs.offset + (eoff + HP + 101) * CE,
                                  [[HP * CE, 100], [1, CE]])
                    nc.scalar.dma_start(out=col, in_=ztile[:100, :])
                for e01 in range(2):
                    ep = 2 * s + e01
                    eoff = e01 * ROWS_EP
                    for blk in range(10):
                        it = apool.tile([128, 1000], f32, tag="tin")
                        nc.scalar.dma_start(
                            out=it[:], in_=fm_in[ep, :, blk * 1000:(blk + 1) * 1000])
                        ot = apool.tile([100, 10 * CE], f32, tag="tout")
                        for g, nb in ((0, 4), (4, 4), (8, 2)):
                            ps = ppool.tile([100, 512], f32, tag="ps")
                            for j in range(nb):
                                nc.tensor.transpose(
                                    out=ps[:, j * CE:(j + 1) * CE],
                                    in_=it[:, (g + j) * 100:(g + j + 1) * 100],
                                    identity=ident[:])
                            nc.scalar.copy(out=ot[:, g * CE:(g + nb) * CE],
                                           in_=ps[:, :nb * CE])
                        dst = bass.AP(
                            fs.tensor,
                            fs.offset + (eoff + (1 + blk * 10) * HP + 1) * CE,
                            [[CE, 100], [HP * CE, 10], [1, CE]])
                        nc.scalar.dma_start(out=dst, in_=ot[:].rearrange(
                            "p (r c) -> p r c", c=CE))

            seg_base = [0]
            for s in range(NSEG):
                seg_base.append(seg_base[-1] + seg_pts[s])

            seg_state = {}

            def phase_p(s):
                base = seg_base[s]
                # ---- Phase B1: coordinate/weight/index pass for the segment
                nt = seg_pts[s] // 128          # total slots in segment
                nt8 = 8 * nt
                sq = bpool.tile([128, nt, 2], f32, tag="sq")
                nc.sync.dma_start(
                    out=sq[:],
                    in_=seq_in[base:base + seg_pts[s], :].rearrange(
                        "(p k) c -> p k c", p=128))
                ct = bpool.tile([128, nt8, 3], f32, tag="ct")
                nc.sync.dma_start(
                    out=ct[:],
                    in_=c16_in[8 * base:8 * (base + seg_pts[s]), :].rearrange(
                        "(p t) c -> p t c", p=128))

                def mapped(src, tag, shape):
                    m = bpool.tile(shape, f32, tag=tag + "m")
                    nc.vector.tensor_scalar(out=m[:], in0=src, scalar1=56.0,
                                            scalar2=None, op0=OP.add)
                    nc.vector.tensor_scalar(out=m[:], in0=m[:], scalar1=R112,
                                            scalar2=100.0, op0=OP.mult,
                                            op1=OP.mult)
                    nc.vector.tensor_scalar(out=m[:], in0=m[:], scalar1=1.0,
                                            scalar2=None, op0=OP.add)
                    return m

                def floor_of(m, tag, shape):
                    ii = bpool.tile(shape, i32, tag=tag + "i")
                    nc.vector.tensor_copy(out=ii[:], in_=m[:])
                    ff = bpool.tile(shape, f32, tag=tag + "f")
                    nc.vector.tensor_copy(out=ff[:], in_=ii[:])
                    gt = bpool.tile(shape, f32, tag=tag + "g")
                    nc.vector.tensor_tensor(out=gt[:], in0=ff[:], in1=m[:],
                                            op=OP.is_gt)
                    nc.vector.tensor_tensor(out=ff[:], in0=ff[:], in1=gt[:],
                                            op=OP.subtract)
                    return ff

                def clip_to(v, hi, tag, shape):
                    nc.vector.tensor_scalar(out=v[:], in0=v[:], scalar1=0.0,
                                            scalar2=hi, op0=OP.max, op1=OP.min)
                    return v

                # ---- index pipeline on replicated [128, 8*nt] tiles
                sh8 = [128, nt8]
                xm16 = mapped(ct[:, :, 0], "x16", sh8)
                ym16 = mapped(ct[:, :, 1], "y16", sh8)
                # idx clip to [0,100] differs from weight clip [0,101] only
                # for exact-integer coords 101 where all 4 weights are 0.
                x1f16 = clip_to(floor_of(xm16, "fx16", sh8), 100.0, "cx16", sh8)
                y1f16 = clip_to(floor_of(ym16, "fy16", sh8), 100.0, "cy16", sh8)
                nc.vector.scalar_tensor_tensor(
                    out=y1f16[:], in0=y1f16[:], scalar=float(HP),
                    in1=x1f16[:], op0=OP.mult, op1=OP.add)
                nc.vector.scalar_tensor_tensor(
                    out=y1f16[:], in0=ct[:, :, 2], scalar=float(ROWS_EP),
                    in1=y1f16[:], op0=OP.mult, op1=OP.add)
                idx1 = bpool.tile(sh8, i16, tag="idx1", bufs=4)
                nc.vector.tensor_copy(out=idx1[:], in_=y1f16[:])

                # ---- weight pipeline on [128, nt] tiles
                shw = [128, nt]
                xm = mapped(sq[:, :, 0], "xw", shw)
                ym = mapped(sq[:, :, 1], "yw", shw)
                flx = floor_of(xm, "fxw", shw)
                fly = floor_of(ym, "fyw", shw)

                def ceil_of(fl, m, tag):
                    lt = bpool.tile(shw, f32, tag=tag + "t")
                    nc.vector.tensor_tensor(out=lt[:], in0=fl[:], in1=m[:],
                                            op=OP.is_lt)
                    nc.vector.tensor_tensor(out=lt[:], in0=fl[:], in1=lt[:],
                                            op=OP.add)
                    return lt

                cex = ceil_of(flx, xm, "cex")
                cey = ceil_of(fly, ym, "cey")
                x1c = clip_to(flx, 101.0, "x1w", shw)
                x2c = clip_to(cex, 101.0, "x2w", shw)
                y1c = clip_to(fly, 101.0, "y1w", shw)
                y2c = clip_to(cey, 101.0, "y2w", shw)

                def sub(p, q, tag):
                    o = bpool.tile(shw, f32, tag=tag)
                    nc.vector.tensor_tensor(out=o[:], in0=p[:], in1=q[:],
                                            op=OP.subtract)
                    return o

                wx1 = sub(x2c, xm, "wx1")   # (x2 - x)
                wx2 = sub(xm, x1c, "wx2")   # (x - x1)
                wy1 = sub(y2c, ym, "wy1")   # (y2 - y)
                wy2 = sub(ym, y1c, "wy2")   # (y - y1)

                # reference pairing: tap(y1,x1)*w11, tap(y1,x2)*w12,
                #                    tap(y2,x1)*w21, tap(y2,x2)*w22
                W1 = bpool.tile([128, nt, 2], f32, tag="W1", bufs=4)
                W2 = bpool.tile([128, nt, 2], f32, tag="W2", bufs=4)
                nc.vector.tensor_tensor(out=W1[:, :, 0], in0=wx1[:],
                                        in1=wy1[:], op=OP.mult)   # w11
                nc.vector.tensor_tensor(out=W1[:, :, 1], in0=wx1[:],
                                        in1=wy2[:], op=OP.mult)   # w12
                nc.vector.tensor_tensor(out=W2[:, :, 0], in0=wx2[:],
                                        in1=wy1[:], op=OP.mult)   # w21
                nc.vector.tensor_tensor(out=W2[:, :, 1], in0=wx2[:],
                                        in1=wy2[:], op=OP.mult)   # w22

                mc = bpool.tile([128, nt, 2], f32, tag="mc")
                nc.vector.tensor_copy(out=mc[:, :, 0], in_=xm[:])
                nc.vector.tensor_copy(out=mc[:, :, 1], in_=ym[:])
                nc.sync.dma_start(
                    out=out_mc[base:base + seg_pts[s], :].rearrange(
                        "(p k) c -> p k c", p=128),
                    in_=mc[:])

                seg_state[s] = (idx1, W1, W2, nt)

            def phase_g(s):
                base = seg_base[s]
                fs = fmp[s][:]
                idx1, W1, W2, nt = seg_state.pop(s)
                # ---- Phase B2: chunked gathers + weighted sum
                in_ap1 = bass.AP(fs.tensor, fs.offset,
                                 [[CE, ROWS_PAIR - 1], [1, 2 * CE]])
                in_ap2 = bass.AP(fs.tensor, fs.offset + HP * CE,
                                 [[CE, ROWS_PAIR - HP - 1], [1, 2 * CE]])
                for ci, csz in enumerate(_chunks_of(seg_pts[s])):
                    n = csz // 128
                    k0 = ci * CSL
                    T1 = tpool.tile([128, n, 2 * CE], f32, tag="T1")
                    nc.gpsimd.dma_gather(
                        out_ap=T1[:], in_ap=in_ap1,
                        idxs_ap=idx1[:, ci * 128: ci * 128 + 8 * n],
                        num_idxs=csz, num_idxs_reg=csz,
                        elem_size=2 * CE, elem_step=CE, single_packet=False)
                    T2 = tpool.tile([128, n, 2 * CE], f32, tag="T2")
                    nc.gpsimd.dma_gather(
                        out_ap=T2[:], in_ap=in_ap2,
                        idxs_ap=idx1[:, ci * 128: ci * 128 + 8 * n],
                        num_idxs=csz, num_idxs_reg=csz,
                        elem_size=2 * CE, elem_step=CE, single_packet=False)

                    def wbroad(W):
                        w = W[:, k0:k0 + n, :]
                        return bass.AP(w.tensor, w.offset,
                                       [list(w.ap[0]), [2, n], [1, 2], [0, CE]])

                    T1v = T1[:].rearrange("p k (h c) -> p k h c", h=2)
                    T2v = T2[:].rearrange("p k (h c) -> p k h c", h=2)
                    nc.vector.tensor_tensor(out=T1v, in0=T1v, in1=wbroad(W1),
                                            op=OP.mult)
                    nc.vector.tensor_tensor(out=T2v, in0=T2v, in1=wbroad(W2),
                                            op=OP.mult)
                    nc.vector.tensor_tensor(out=T1v, in0=T1v, in1=T2v,
                                            op=OP.add)
                    of = prpool.tile([128, n, CE], f32, tag="of")
                    nc.vector.tensor_tensor(out=of[:], in0=T1v[:, :, 0, :],
                                            in1=T1v[:, :, 1, :], op=OP.add)
                    # row(p, k) = base + p*nt + k0 + k
                    dst = bass.AP(out_feat.tensor,
                                  out_feat.offset + (base + k0) * CE,
                                  [[nt * CE, 128], [CE, n], [1, CE]])
                    nc.sync.dma_start(out=dst, in_=of[:])

            # schedule: transposes and coordinate pipelines run ahead of
            # the serial gather stream so the Pool engine never starves
            phase_a(0)
            phase_p(0)
            phase_a(1)
            phase_p(1)
            phase_g(0)
            phase_a(2)
            phase_p(2)
            phase_g(1)
            phase_a(3)
            phase_p(3)
            phase_g(2)
            phase_g(3)

    nc.compile()
    return nc


@functools.lru_cache(maxsize=4)
def _compiled(seg_pts, oomv):
    return _build_program(seg_pts, oomv)


def kernel(episode_idx, sequence, feature_map, oom_val):
    _install_axon_profile_hook()
    from concourse.bass_utils import run_bass_kernel_spmd

    ep = np.asarray(episode_idx).astype(np.int64)           # [A]
    seq = np.ascontiguousarray(np.asarray(sequence, dtype=np.float32))
    fm = np.ascontiguousarray(np.asarray(feature_map, dtype=np.float32))
    oomv = float(np.float32(oom_val))
    a_total, td = seq.shape[0], seq.shape[1]

    core_of = ep // EPC
    seg_of = (ep % EPC) // 2
    lep_of = (ep % 2).astype(np.float32)

    agents = [[[] for _ in range(NSEG)] for _ in range(NCORES)]
    for a in range(a_total):
        agents[int(core_of[a])][int(seg_of[a])].append(a)

    seg_pts = []
    for s in range(NSEG):
        mx = max(len(agents[c][s]) * td for c in range(NCORES))
        mx = max(mx, 1)
        seg_pts.append(int(-(-mx // 128) * 128))
    seg_pts = tuple(seg_pts)
    npad = sum(seg_pts)

    nc = _compiled(seg_pts, oomv)

    tdr = np.arange(td, dtype=np.int64)
    in_maps = []
    srcs = []
    for c in range(NCORES):
        rx = np.zeros(npad, np.float32)
        ry = np.zeros(npad, np.float32)
        rl = np.zeros(npad, np.float32)
        rsrc = np.full(npad, -1, np.int64)
        base = 0
        for s in range(NSEG):
            ags = np.asarray(agents[c][s], dtype=np.int64)
            npts = len(ags) * td
            if npts:
                rx[base:base + npts] = seq[ags, :, 0].ravel()
                ry[base:base + npts] = seq[ags, :, 1].ravel()
                rl[base:base + npts] = np.repeat(lep_of[ags], td)
                rsrc[base:base + npts] = (ags[:, None] * td + tdr[None, :]).ravel()
            base += seg_pts[s]
        # gather-order (16-wrap, replicated) blocks per segment
        c16_parts = []
        base = 0
        for s in range(NSEG):
            spts = seg_pts[s]
            nt = spts // 128
            i = np.arange(spts, dtype=np.int64)      # segment-linear gather order
            cidx = i // CHUNK
            j = i % CHUNK
            srow = base + (j % 128) * nt + cidx * CSL + (j // 128)
            blk = np.stack([rx[srow], ry[srow], rl[srow]], axis=1)   # [spts,3]
            t16 = blk.reshape(spts // 16, 16, 3).transpose(1, 0, 2)  # [16,8nt,3]
            c16_parts.append(np.tile(t16, (8, 1, 1)).reshape(-1, 3))
            base += spts
        in_maps.append({
            "fm": fm[c * EPC:(c + 1) * EPC].reshape(EPC, CE, 10000),
            "seqw": np.stack([rx, ry], axis=1),
            "c16": np.ascontiguousarray(np.concatenate(c16_parts, axis=0)),
        })
        srcs.append(rsrc)

    res = run_bass_kernel_spmd(nc, in_maps, core_ids=list(range(NCORES)))

    lf = np.zeros((a_total * td, CE), np.float32)
    mc = np.zeros((a_total * td, 2), np.float32)
    for c in range(NCORES):
        rsrc = srcs[c]
        v = rsrc >= 0
        lf[rsrc[v]] = res.results[c]["out_feat"][v]
        mc[rsrc[v]] = res.results[c]["out_mc"][v]
    return lf.reshape(a_total, td, CE), mc.reshape(a_total, td, 2)
